# revision 4
# baseline (speedup 1.0000x reference)
"""Trainium2 Bass kernel for BinaryRelativePositionEmbedding.

Math: out[b,h,l,m] = q[b,h,l,:] . rp[m,:],  rp = bits @ emb, where
bits[m,:] are the 12 two's-complement bits of position (m - L + 1).

Key identity: out[l, m] = sum_b bits[m,b] * s[l,b] with s = q @ emb^T
(rank 12).  The pattern v(m) = (m - (L-1)) & 4095 ranges over all 12-bit
values except 2048, so each row-tile of the output is a subset-sum table
over the 12 per-row scalars s[l, :], built with doubling steps on the
vector engine.  The table is laid out rotated by 2048 so the final
output row is the single contiguous slice U[:, 1:4096]:
    U[:, 2048+w] = subset-sum of bits 0..10 over w   (w in [0,2048))
    U[:, c]      = U[:, 2048+c] + s_11               (c in [0,2048))
    => U[:, 1+m] = T[(m + 2049) & 4095] = out[:, m]  (m in [0,4095))
giving one 16380-byte contiguous DMA descriptor per output row.

Output DMAs alternate between the two HWDGE rings per batch, with the
table build deliberately DVE-paced so the rings are almost never
concurrently active: a lone 16-engine DMA stream already saturates the
SBUF AXI ports, two concurrently-active rings make every engine
round-robin between them at packet granularity (+20-35%/descriptor),
and indefinitely-long single-ring runs make SDMA engine 15 degrade
progressively after ~200us.  The table build stays entirely on the
vector engine — scalar-engine SBUF ops contend with DMA reads.

Sharding: data-parallel over the 32 (b,h) pairs, 4 per NeuronCore.
"""

import os
import sys

import numpy as np

if "/opt/trn_rl_repo" not in sys.path:
    sys.path.insert(0, "/opt/trn_rl_repo")

import concourse.bass as bass  # noqa: E402
import concourse.mybir as mybir  # noqa: E402
from concourse import bacc, tile  # noqa: E402
from concourse.bass_utils import run_bass_kernel_spmd  # noqa: E402

F32 = mybir.dt.float32

B, H, L, D = 2, 16, 2048, 64
NB = 12                  # bits per position
M = 2 * L - 1            # 4095 relative positions
NCORES = 8
PAIRS = B * H            # 32
PPC = PAIRS // NCORES    # 4 (b,h) pairs per core
ROWS = PPC * L           # 8192 output rows per core


LAST_EXEC_TIME_NS = None


def _build_nc():
    nc = bacc.Bacc(None)
    qT = nc.declare_dram_parameter("qT", [D, ROWS], F32, isOutput=False)
    embT = nc.declare_dram_parameter("embT", [D, NB], F32, isOutput=False)
    out = nc.declare_dram_parameter("out", [ROWS, M], F32, isOutput=True)

    tiles = [(i * 128, 128) for i in range(ROWS // 128)]
    nt = len(tiles)
    NBATCH = 2
    batches = [[i, i + 1] for i in range(0, nt, 2)]

    # input chunks: 8 row-tiles' worth of qT each
    chunks = []
    for g0 in range(0, nt, 8):
        grp = tiles[g0 : g0 + 8]
        c0 = grp[0][0]
        csz = grp[-1][0] + grp[-1][1] - c0
        chunks.append((c0, csz))

    with tile.TileContext(nc) as tc:
        with (
            tc.tile_pool(name="const", bufs=1) as cpool,
            tc.tile_pool(name="psum", bufs=8, space="PSUM") as ppool,
            tc.tile_pool(name="tab", bufs=3) as tpool,
        ):
            embt_sb = cpool.tile([D, NB], F32)
            s_sb = cpool.tile([128, nt * NB], F32)
            qt_chunks = [
                cpool.tile([D, csz], F32, name=f"qt{g}", tag=f"qt{g}")
                for g, (_, csz) in enumerate(chunks)
            ]

            nc.scalar.dma_start(out=embt_sb[:], in_=embT[:])
            for g, (c0, csz) in enumerate(chunks):
                nc.scalar.dma_start(out=qt_chunks[g][:], in_=qT[:, c0 : c0 + csz])

            # Interleave per group of 8 row-tiles: matmul group g -> copy ->
            # build+drain its 4 batches.  With copies in DVE program order
            # right before their builds, the first output DMA issues ~16us in
            # instead of waiting for all 64 matmuls (~33us).
            for g0 in range(0, nt, 8):
                grp = list(range(g0, min(g0 + 8, nt)))
                ps = ppool.tile([128, 8 * NB], F32, name="ps", tag="ps")
                for j, t in enumerate(grp):
                    r0, nr = tiles[t]
                    ci = t // 8
                    off = r0 - chunks[ci][0]
                    nc.tensor.matmul(
                        ps[0:nr, j * NB : (j + 1) * NB],
                        lhsT=qt_chunks[ci][:, off : off + nr],
                        rhs=embt_sb[:],
                        start=True,
                        stop=True,
                    )
                # s[l, b] = q[l, :] . emb[b, :]
                nc.vector.tensor_copy(
                    out=s_sb[:, g0 * NB : (g0 + len(grp)) * NB],
                    in_=ps[:, : len(grp) * NB],
                )

                for batch in [b for b in batches if g0 <= b[0] < g0 + 8]:
                    nr = tiles[batch[0]][1]
                    nb = len(batch)
                    U = tpool.tile([128, nb * 4096], F32, name="U", tag="U")
                    for j, ti in enumerate(batch):
                        sb = ti * NB
                        base = j * 4096
                        hi = base + 2048
                        nc.vector.memset(U[0:nr, hi : hi + 1], 0.0)
                        nc.vector.tensor_copy(
                            out=U[0:nr, hi + 1 : hi + 2], in_=s_sb[0:nr, sb : sb + 1]
                        )
                        for k in range(1, NB - 1):
                            nc.vector.tensor_scalar_add(
                                U[0:nr, hi + 2**k : hi + 2 ** (k + 1)],
                                U[0:nr, hi : hi + 2**k],
                                s_sb[0:nr, sb + k : sb + k + 1],
                            )
                        nc.vector.tensor_scalar_add(
                            U[0:nr, base : base + 2048],
                            U[0:nr, hi : hi + 2048],
                            s_sb[0:nr, sb + NB - 1 : sb + NB],
                        )
                    r0 = tiles[batch[0]][0]
                    src = U[0:nr].rearrange("p (j c) -> p j c", j=nb)[:, :, 1:4096]
                    dst = out[r0 : r0 + nb * nr, :].rearrange("(j p) m -> p j m", p=nr)
                    # single ring for every output DMA: when two rings hold
                    # backlog concurrently, each SDMA engine round-robins
                    # between them at packet granularity and per-packet time
                    # degrades 629ns -> 824ns (+31%).  One FIFO ring keeps
                    # every engine on one stream at full rate.
                    nc.sync.dma_start(out=dst, in_=src)

    nc.finalize()
    return nc


def _install_trace_shim():
    """Make run_bass_kernel_spmd(trace=True) work under axon in this
    container: provide antenv.axon_hooks backed by ctypes calls into
    libaxon_pjrt.so, and skip the S3 artifact upload."""
    import contextlib
    import ctypes
    import types

    import antenv
    from concourse import bass_utils

    if getattr(antenv, "axon_hooks", None) is not None:
        return

    def _ntff_profile_via_ctypes(so_path):
        lib = ctypes.CDLL(so_path)
        if not hasattr(lib, "axon_start_nrt_profile"):
            return None
        lib.axon_start_nrt_profile.argtypes = [
            ctypes.POINTER(ctypes.c_int64),
            ctypes.c_size_t,
        ]
        lib.axon_start_nrt_profile.restype = ctypes.c_int64
        lib.axon_stop_nrt_profile.argtypes = [ctypes.c_char_p]
        lib.axon_stop_nrt_profile.restype = ctypes.c_int64

        @contextlib.contextmanager
        def _hook(output_dir, device_ids):
            import jax

            jax.devices()
            if device_ids:
                ids = (ctypes.c_int64 * len(device_ids))(*device_ids)
                rc = lib.axon_start_nrt_profile(ids, len(device_ids))
            else:
                rc = lib.axon_start_nrt_profile(None, 0)
            if rc != 0:
                raise RuntimeError(f"axon_start_nrt_profile rc={rc}")
            try:
                yield
            finally:
                n = lib.axon_stop_nrt_profile(str(output_dir).encode())
                print(f"trace shim: {n} ntff file(s) in {output_dir}", file=sys.stderr)

        return _hook

    mod = types.ModuleType("antenv.axon_hooks")
    state = {"hook": _ntff_profile_via_ctypes("/opt/axon/libaxon_pjrt.so")}
    mod.set_axon_ntff_profile_hook = lambda h: state.__setitem__("hook", h)
    mod.get_axon_ntff_profile_hook = lambda: state["hook"]
    sys.modules["antenv.axon_hooks"] = mod
    antenv.axon_hooks = mod
    bass_utils.upload_artifacts = lambda tmpdir: f"local://{tmpdir}"


def kernel(q, k, emb):
    global LAST_EXEC_TIME_NS
    trace = os.environ.get("KERNEL_TRACE", "") == "1"
    if trace:
        _install_trace_shim()

    nc = _build_nc()

    qr = np.asarray(q, dtype=np.float32).reshape(PAIRS, L, D)
    embT = np.ascontiguousarray(np.asarray(emb, dtype=np.float32).T)
    in_maps = []
    for c in range(NCORES):
        qc = qr[c * PPC : (c + 1) * PPC]  # [PPC, L, D]
        qTc = np.ascontiguousarray(qc.transpose(2, 0, 1).reshape(D, ROWS))
        in_maps.append({"qT": qTc, "embT": embT})

    res = run_bass_kernel_spmd(nc, in_maps, core_ids=list(range(NCORES)), trace=trace)
    LAST_EXEC_TIME_NS = res.exec_time_ns

    out = np.empty((PAIRS, L, M), np.float32)
    for c in range(NCORES):
        out[c * PPC : (c + 1) * PPC] = res.results[c]["out"].reshape(PPC, L, M)
    return out.reshape(B, H, L, M)



# revision 6
# speedup vs baseline: 1.0603x; 1.0603x over previous
"""Trainium2 Bass kernel for BinaryRelativePositionEmbedding.

Math: out[b,h,l,m] = q[b,h,l,:] . rp[m,:],  rp = bits @ emb, where
bits[m,:] are the 12 two's-complement bits of position (m - L + 1).

Key identity: out[l, m] = sum_b bits[m,b] * s[l,b] with s = q @ emb^T
(rank 12).  The pattern v(m) = (m - (L-1)) & 4095 ranges over all 12-bit
values except 2048, so each row-tile of the output is a subset-sum table
over the 12 per-row scalars s[l, :], built with doubling steps on the
vector engine.  The table is laid out rotated by 2048 so the final
output row is the single contiguous slice U[:, 1:4096]:
    U[:, 2048+w] = subset-sum of bits 0..10 over w   (w in [0,2048))
    U[:, c]      = U[:, 2048+c] + s_11               (c in [0,2048))
    => U[:, 1+m] = T[(m + 2049) & 4095] = out[:, m]  (m in [0,4095))
giving one 16380-byte contiguous DMA descriptor per output row.

All output DMAs go on ONE HWDGE ring (nc.sync): when two rings hold
backlog concurrently, each SDMA engine round-robins between them at
packet granularity and per-packet time degrades 629ns -> 824ns (+31%).
A single FIFO ring sustains ~425 GB/s (617ns per 16380B packet, 97.6%
of the 435 GB/s SBUF-AXI fabric ceiling) for the whole drain.

Startup: PSUM groups are split [1,1,2,4,8,...] with the tiny s-copies
on the SCALAR engine, so the first table build depends only on tile 0's
matmul and the DVE critical path is builds-only (the Tile scheduler
otherwise hoists copies + their matmul waits between early builds,
starving the ring).  PSUM stays at bufs=2: freeing it (bufs=8) lets the
scheduler front-load all matmuls, which empirically locks the DMA into
a degraded ~860ns/packet regime (+22%% exec).

Sharding: data-parallel over the 32 (b,h) pairs, 4 per NeuronCore.
"""

import os
import sys

import numpy as np

if "/opt/trn_rl_repo" not in sys.path:
    sys.path.insert(0, "/opt/trn_rl_repo")

import concourse.bass as bass  # noqa: E402
import concourse.mybir as mybir  # noqa: E402
from concourse import bacc, tile  # noqa: E402
from concourse.bass_utils import run_bass_kernel_spmd  # noqa: E402

F32 = mybir.dt.float32

B, H, L, D = 2, 16, 2048, 64
NB = 12                  # bits per position
M = 2 * L - 1            # 4095 relative positions
NCORES = 8
PAIRS = B * H            # 32
PPC = PAIRS // NCORES    # 4 (b,h) pairs per core
ROWS = PPC * L           # 8192 output rows per core
NT = ROWS // 128         # 64 row-tiles

# PSUM sub-groups: first tiles get their own psum tile + copy so the
# first build only waits on one matmul.
GROUPS = [1, 1, 2, 4] + [8] * 7


LAST_EXEC_TIME_NS = None


def _build_nc():
    nc = bacc.Bacc(None)
    qT = nc.declare_dram_parameter("qT", [D, ROWS], F32, isOutput=False)
    embT = nc.declare_dram_parameter("embT", [D, NB], F32, isOutput=False)
    out = nc.declare_dram_parameter("out", [ROWS, M], F32, isOutput=True)

    # input chunks: first tile alone so matmul 0 starts ASAP, then the
    # next 7 tiles, then 8-tile chunks.
    chunk_tiles = [1, 7] + [8] * 7
    chunks = []
    c0 = 0
    for n in chunk_tiles:
        chunks.append((c0, n * 128))
        c0 += n * 128

    def chunk_of(tile_idx):
        c0 = 0
        for ci, n in enumerate(chunk_tiles):
            if tile_idx < c0 + n:
                return ci, c0
            c0 += n
        raise AssertionError

    with tile.TileContext(nc) as tc:
        with (
            tc.tile_pool(name="const", bufs=1) as cpool,
            tc.tile_pool(name="psum", bufs=2, space="PSUM") as ppool,
            tc.tile_pool(name="tab", bufs=6) as tpool,
        ):
            embt_sb = cpool.tile([D, NB], F32)
            s_sb = cpool.tile([128, NT * NB], F32)
            qt_chunks = [
                cpool.tile([D, csz], F32, name=f"qt{g}", tag=f"qt{g}")
                for g, (_, csz) in enumerate(chunks)
            ]

            nc.scalar.dma_start(out=embt_sb[:], in_=embT[:])
            for g, (c0, csz) in enumerate(chunks):
                nc.scalar.dma_start(out=qt_chunks[g][:], in_=qT[:, c0 : c0 + csz])

            t0 = 0
            for grp_n in GROUPS:
                grp = list(range(t0, t0 + grp_n))
                ps = ppool.tile([128, grp_n * NB], F32, name="ps", tag="ps")
                for j, t in enumerate(grp):
                    ci, ct0 = chunk_of(t)
                    off = (t - ct0) * 128
                    nc.tensor.matmul(
                        ps[:, j * NB : (j + 1) * NB],
                        lhsT=qt_chunks[ci][:, off : off + 128],
                        rhs=embt_sb[:],
                        start=True,
                        stop=True,
                    )
                # s[l, b] = q[l, :] . emb[b, :]; copy on the scalar engine
                # keeps the DVE stream builds-only.
                nc.scalar.copy(
                    out=s_sb[:, t0 * NB : (t0 + grp_n) * NB],
                    in_=ps[:, : grp_n * NB],
                )

                for ti in grp:
                    sb = ti * NB
                    U = tpool.tile([128, 4096], F32, name="U", tag="U")
                    hi = 2048
                    nc.vector.memset(U[:, hi : hi + 1], 0.0)
                    nc.vector.tensor_copy(
                        out=U[:, hi + 1 : hi + 2], in_=s_sb[:, sb : sb + 1]
                    )
                    for k in range(1, NB - 1):
                        nc.vector.tensor_scalar_add(
                            U[:, hi + 2**k : hi + 2 ** (k + 1)],
                            U[:, hi : hi + 2**k],
                            s_sb[:, sb + k : sb + k + 1],
                        )
                    nc.vector.tensor_scalar_add(
                        U[:, 0:2048],
                        U[:, hi : hi + 2048],
                        s_sb[:, sb + NB - 1 : sb + NB],
                    )
                    r0 = ti * 128
                    nc.sync.dma_start(
                        out=out[r0 : r0 + 128, :], in_=U[:, 1:4096]
                    )
                t0 += grp_n

    nc.finalize()
    return nc


def _install_trace_shim():
    """Make run_bass_kernel_spmd(trace=True) work under axon in this
    container: provide antenv.axon_hooks backed by ctypes calls into
    libaxon_pjrt.so, and skip the S3 artifact upload."""
    import contextlib
    import ctypes
    import types

    import antenv
    from concourse import bass_utils

    if getattr(antenv, "axon_hooks", None) is not None:
        return

    def _ntff_profile_via_ctypes(so_path):
        lib = ctypes.CDLL(so_path)
        if not hasattr(lib, "axon_start_nrt_profile"):
            return None
        lib.axon_start_nrt_profile.argtypes = [
            ctypes.POINTER(ctypes.c_int64),
            ctypes.c_size_t,
        ]
        lib.axon_start_nrt_profile.restype = ctypes.c_int64
        lib.axon_stop_nrt_profile.argtypes = [ctypes.c_char_p]
        lib.axon_stop_nrt_profile.restype = ctypes.c_int64

        @contextlib.contextmanager
        def _hook(output_dir, device_ids):
            import jax

            jax.devices()
            if device_ids:
                ids = (ctypes.c_int64 * len(device_ids))(*device_ids)
                rc = lib.axon_start_nrt_profile(ids, len(device_ids))
            else:
                rc = lib.axon_start_nrt_profile(None, 0)
            if rc != 0:
                raise RuntimeError(f"axon_start_nrt_profile rc={rc}")
            try:
                yield
            finally:
                n = lib.axon_stop_nrt_profile(str(output_dir).encode())
                print(f"trace shim: {n} ntff file(s) in {output_dir}", file=sys.stderr)

        return _hook

    mod = types.ModuleType("antenv.axon_hooks")
    state = {"hook": _ntff_profile_via_ctypes("/opt/axon/libaxon_pjrt.so")}
    mod.set_axon_ntff_profile_hook = lambda h: state.__setitem__("hook", h)
    mod.get_axon_ntff_profile_hook = lambda: state["hook"]
    sys.modules["antenv.axon_hooks"] = mod
    antenv.axon_hooks = mod
    bass_utils.upload_artifacts = lambda tmpdir: f"local://{tmpdir}"


def kernel(q, k, emb):
    global LAST_EXEC_TIME_NS
    trace = os.environ.get("KERNEL_TRACE", "") == "1"
    if trace:
        _install_trace_shim()

    nc = _build_nc()

    qr = np.asarray(q, dtype=np.float32).reshape(PAIRS, L, D)
    embT = np.ascontiguousarray(np.asarray(emb, dtype=np.float32).T)
    in_maps = []
    for c in range(NCORES):
        qc = qr[c * PPC : (c + 1) * PPC]  # [PPC, L, D]
        qTc = np.ascontiguousarray(qc.transpose(2, 0, 1).reshape(D, ROWS))
        in_maps.append({"qT": qTc, "embT": embT})

    res = run_bass_kernel_spmd(nc, in_maps, core_ids=list(range(NCORES)), trace=trace)
    LAST_EXEC_TIME_NS = res.exec_time_ns

    out = np.empty((PAIRS, L, M), np.float32)
    for c in range(NCORES):
        out[c * PPC : (c + 1) * PPC] = res.results[c]["out"].reshape(PPC, L, M)
    return out.reshape(B, H, L, M)


# revision 10
# speedup vs baseline: 1.2569x; 1.1854x over previous
"""Trainium2 Bass kernel for BinaryRelativePositionEmbedding.

Math: out[b,h,l,m] = q[b,h,l,:] . rp[m,:],  rp = bits @ emb, where
bits[m,:] are the 12 two's-complement bits of position (m - L + 1).

Key identity: out[l, m] = sum_b bits[m,b] * s[l,b] with s = q @ emb^T
(rank 12).  The pattern v(m) = (m - (L-1)) & 4095 ranges over all 12-bit
values except 2048, so each row-tile of the output is a subset-sum table
over the 12 per-row scalars s[l, :], built with doubling steps on the
vector engine.  The table is laid out rotated by 2048 so the final
output row is the single contiguous slice U[:, 1:4096]:
    U[:, 2048+w] = subset-sum of bits 0..10 over w   (w in [0,2048))
    U[:, c]      = U[:, 2048+c] + s_11               (c in [0,2048))
    => U[:, 1+m] = T[(m + 2049) & 4095] = out[:, m]  (m in [0,4095))
giving one 16380-byte contiguous DMA descriptor per output row.

All output DMAs go on ONE HWDGE ring (nc.sync): when two rings hold
backlog concurrently, each SDMA engine round-robins between them at
packet granularity and per-packet time degrades 629ns -> 824ns (+31%).
A single FIFO ring sustains ~425 GB/s (617ns per 16380B packet, 97.6%
of the 435 GB/s SBUF-AXI fabric ceiling) for the whole drain.

Startup: PSUM groups are split [1,1,2,4,8,...] with the tiny s-copies
on the SCALAR engine, so the first table build depends only on tile 0's
matmul and the DVE critical path is builds-only (the Tile scheduler
otherwise hoists copies + their matmul waits between early builds,
starving the ring).  PSUM stays at bufs=2: freeing it (bufs=8) lets the
scheduler front-load all matmuls, which empirically locks the DMA into
a degraded ~860ns/packet regime (+22%% exec).

Sharding: data-parallel over the 32 (b,h) pairs, 4 per NeuronCore.
"""

import os
import sys

import numpy as np

if "/opt/trn_rl_repo" not in sys.path:
    sys.path.insert(0, "/opt/trn_rl_repo")

import concourse.bass as bass  # noqa: E402
import concourse.mybir as mybir  # noqa: E402
from concourse import bacc, tile  # noqa: E402
from concourse.bass_utils import run_bass_kernel_spmd  # noqa: E402

F32 = mybir.dt.float32

B, H, L, D = 2, 16, 2048, 64
NB = 12                  # bits per position
M = 2 * L - 1            # 4095 relative positions
NCORES = 8
PAIRS = B * H            # 32
PPC = PAIRS // NCORES    # 4 (b,h) pairs per core
ROWS = PPC * L           # 8192 output rows per core
NT = ROWS // 128         # 64 row-tiles

# PSUM sub-groups: first tiles get their own psum tile + copy so the
# first build only waits on two matmuls.  Builds go in 2-tile batches:
# per-batch fixed overhead (semaphore + sequencer dead time) is ~1.2us
# regardless of batch size, so 1-tile batches make the DVE the pacer
# (+56us vector time, exec 411us vs 358us).
GROUPS = [2, 2, 4] + [8] * 7


LAST_EXEC_TIME_NS = None


def _build_nc():
    nc = bacc.Bacc(None)
    qT = nc.declare_dram_parameter("qT", [D, ROWS], F32, isOutput=False)
    embT = nc.declare_dram_parameter("embT", [D, NB], F32, isOutput=False)
    out = nc.declare_dram_parameter("out", [ROWS, M], F32, isOutput=True)

    # input chunks: first two tiles alone so the first matmuls start
    # ASAP, then the next 6 tiles, then 8-tile chunks.
    chunk_tiles = [2, 6] + [8] * 7
    chunks = []
    c0 = 0
    for n in chunk_tiles:
        chunks.append((c0, n * 128))
        c0 += n * 128

    def chunk_of(tile_idx):
        c0 = 0
        for ci, n in enumerate(chunk_tiles):
            if tile_idx < c0 + n:
                return ci, c0
            c0 += n
        raise AssertionError

    with tile.TileContext(nc) as tc:
        with (
            tc.tile_pool(name="const", bufs=1) as cpool,
            tc.tile_pool(name="psum", bufs=2, space="PSUM") as ppool,
            tc.tile_pool(name="tab", bufs=3) as tpool,
        ):
            embt_sb = cpool.tile([D, NB], F32)
            s_sb = cpool.tile([128, NT * NB], F32)
            qt_chunks = [
                cpool.tile([D, csz], F32, name=f"qt{g}", tag=f"qt{g}")
                for g, (_, csz) in enumerate(chunks)
            ]

            nc.scalar.dma_start(out=embt_sb[:], in_=embT[:])
            for g, (c0, csz) in enumerate(chunks):
                nc.scalar.dma_start(out=qt_chunks[g][:], in_=qT[:, c0 : c0 + csz])

            t0 = 0
            for grp_n in GROUPS:
                grp = list(range(t0, t0 + grp_n))
                ps = ppool.tile([128, grp_n * NB], F32, name="ps", tag="ps")
                for j, t in enumerate(grp):
                    ci, ct0 = chunk_of(t)
                    off = (t - ct0) * 128
                    nc.tensor.matmul(
                        ps[:, j * NB : (j + 1) * NB],
                        lhsT=qt_chunks[ci][:, off : off + 128],
                        rhs=embt_sb[:],
                        start=True,
                        stop=True,
                    )
                # s[l, b] = q[l, :] . emb[b, :]; copy on the scalar engine
                # keeps the DVE stream builds-only.
                nc.scalar.copy(
                    out=s_sb[:, t0 * NB : (t0 + grp_n) * NB],
                    in_=ps[:, : grp_n * NB],
                )

                for b0 in range(t0, t0 + grp_n, 2):
                    batch = [b0, b0 + 1]
                    U = tpool.tile([128, 2 * 4096], F32, name="U", tag="U")
                    for j, ti in enumerate(batch):
                        sb = ti * NB
                        base = j * 4096
                        hi = base + 2048
                        nc.vector.memset(U[:, hi : hi + 1], 0.0)
                        nc.vector.tensor_copy(
                            out=U[:, hi + 1 : hi + 2], in_=s_sb[:, sb : sb + 1]
                        )
                        for k in range(1, NB - 1):
                            nc.vector.tensor_scalar_add(
                                U[:, hi + 2**k : hi + 2 ** (k + 1)],
                                U[:, hi : hi + 2**k],
                                s_sb[:, sb + k : sb + k + 1],
                            )
                        nc.vector.tensor_scalar_add(
                            U[:, base : base + 2048],
                            U[:, hi : hi + 2048],
                            s_sb[:, sb + NB - 1 : sb + NB],
                        )
                    r0 = b0 * 128
                    src = U.rearrange("p (j c) -> p j c", j=2)[:, :, 1:4096]
                    dst = out[r0 : r0 + 256, :].rearrange("(j p) m -> p j m", p=128)
                    nc.sync.dma_start(out=dst, in_=src)
                t0 += grp_n

    nc.finalize()
    return nc


def _install_trace_shim():
    """Make run_bass_kernel_spmd(trace=True) work under axon in this
    container: provide antenv.axon_hooks backed by ctypes calls into
    libaxon_pjrt.so, and skip the S3 artifact upload."""
    import contextlib
    import ctypes
    import types

    import antenv
    from concourse import bass_utils

    if getattr(antenv, "axon_hooks", None) is not None:
        return

    def _ntff_profile_via_ctypes(so_path):
        lib = ctypes.CDLL(so_path)
        if not hasattr(lib, "axon_start_nrt_profile"):
            return None
        lib.axon_start_nrt_profile.argtypes = [
            ctypes.POINTER(ctypes.c_int64),
            ctypes.c_size_t,
        ]
        lib.axon_start_nrt_profile.restype = ctypes.c_int64
        lib.axon_stop_nrt_profile.argtypes = [ctypes.c_char_p]
        lib.axon_stop_nrt_profile.restype = ctypes.c_int64

        @contextlib.contextmanager
        def _hook(output_dir, device_ids):
            import jax

            jax.devices()
            if device_ids:
                ids = (ctypes.c_int64 * len(device_ids))(*device_ids)
                rc = lib.axon_start_nrt_profile(ids, len(device_ids))
            else:
                rc = lib.axon_start_nrt_profile(None, 0)
            if rc != 0:
                raise RuntimeError(f"axon_start_nrt_profile rc={rc}")
            try:
                yield
            finally:
                n = lib.axon_stop_nrt_profile(str(output_dir).encode())
                print(f"trace shim: {n} ntff file(s) in {output_dir}", file=sys.stderr)

        return _hook

    mod = types.ModuleType("antenv.axon_hooks")
    state = {"hook": _ntff_profile_via_ctypes("/opt/axon/libaxon_pjrt.so")}
    mod.set_axon_ntff_profile_hook = lambda h: state.__setitem__("hook", h)
    mod.get_axon_ntff_profile_hook = lambda: state["hook"]
    sys.modules["antenv.axon_hooks"] = mod
    antenv.axon_hooks = mod
    bass_utils.upload_artifacts = lambda tmpdir: f"local://{tmpdir}"


def kernel(q, k, emb):
    global LAST_EXEC_TIME_NS
    trace = os.environ.get("KERNEL_TRACE", "") == "1"
    if trace:
        _install_trace_shim()

    nc = _build_nc()

    qr = np.asarray(q, dtype=np.float32).reshape(PAIRS, L, D)
    embT = np.ascontiguousarray(np.asarray(emb, dtype=np.float32).T)
    in_maps = []
    for c in range(NCORES):
        qc = qr[c * PPC : (c + 1) * PPC]  # [PPC, L, D]
        qTc = np.ascontiguousarray(qc.transpose(2, 0, 1).reshape(D, ROWS))
        in_maps.append({"qT": qTc, "embT": embT})

    res = run_bass_kernel_spmd(nc, in_maps, core_ids=list(range(NCORES)), trace=trace)
    LAST_EXEC_TIME_NS = res.exec_time_ns

    out = np.empty((PAIRS, L, M), np.float32)
    for c in range(NCORES):
        out[c * PPC : (c + 1) * PPC] = res.results[c]["out"].reshape(PPC, L, M)
    return out.reshape(B, H, L, M)
